# revision 1
# baseline (speedup 1.0000x reference)
"""Distributed Trainium2 kernel for nn_Attn_77970836292156.

Cross-attention block: fused QKV projection + per-head RMSNorm + RoPE +
bf16 SDPA (4096 keys = 2048 self + 2048 cross) + output projection.

Sharding: tensor-parallel on heads. 16 heads / 8 cores = 2 heads per core.
W_qkv / W_ckv column-sharded by head; every core holds full x, y (transposed,
bf16). Attention runs fully local per core in a transposed layout
(head-dims on partitions, positions on the free axis), producing
OT [128 dims, 2048 q]. An AllToAll converts head-sharding -> sequence-
sharding, then each core applies the full W_out to its 256-row slice
(row-sharded matmul accumulated over all 1024 dims), so no AllReduce is
needed and the output projection's reduction happens on the TensorEngine.

Softmax is computed in a transposed "scores^T" layout [keys, q] so that the
denominator comes for free out of the PV matmul (a ones-column appended to V),
and exp() uses max-free softmax (scores are ~N(0,1) after the 1/8 scale).
RMSNorm scales and RoPE tables (with g_q/g_k/g_ck folded in host-side) are
applied via DVE/GPSIMD elementwise ops; per-row 1/sqrt(ms+eps) is built with
Ln/Exp from one ACT table set to avoid activation-table reloads.
"""

import os

import numpy as np
import ml_dtypes

import concourse.bass as bass
import concourse.tile as tile
from concourse import bacc, mybir
from concourse.bass_utils import run_bass_kernel_spmd

BF16 = mybir.dt.bfloat16
F32 = mybir.dt.float32

# Problem constants (hardcoded per spec).
N = 2048        # query positions
M = 2048        # cross positions
NK = N + M      # total keys
D = 1024        # model dim
H = 16          # heads
DH = 64         # head dim
HL = 2          # heads per core
DL = HL * DH    # local head dims = 128
F = 1024        # input features
P = 128
NCORES = 8
EPS = 1e-6
ROPE_BASE = 10000.0
SCALE = 0.125   # 1/sqrt(64)

LAST_RESULT = None  # test harness reads exec_time_ns from here


def build_nc():
    nc = bacc.Bacc()

    # ---------------- DRAM parameters ----------------
    xT = nc.declare_dram_parameter("xT", [F, N], BF16, isOutput=False)
    yT = nc.declare_dram_parameter("yT", [F, M], BF16, isOutput=False)
    wq = nc.declare_dram_parameter("wq", [F, DL], BF16, isOutput=False)
    wk = nc.declare_dram_parameter("wk", [F, DL], BF16, isOutput=False)
    wv = nc.declare_dram_parameter("wv", [F, DL], BF16, isOutput=False)
    wck = nc.declare_dram_parameter("wck", [F, DL], BF16, isOutput=False)
    wcv = nc.declare_dram_parameter("wcv", [F, DL], BF16, isOutput=False)
    wo = nc.declare_dram_parameter("wo", [D, D], BF16, isOutput=False)
    bo = nc.declare_dram_parameter("bo", [1, D], BF16, isOutput=False)
    cq = nc.declare_dram_parameter("cq", [P, N], BF16, isOutput=False)
    sq = nc.declare_dram_parameter("sq", [P, N], BF16, isOutput=False)
    ckc = nc.declare_dram_parameter("ckc", [P, NK], BF16, isOutput=False)
    cks = nc.declare_dram_parameter("cks", [P, NK], BF16, isOutput=False)
    hmask = nc.declare_dram_parameter("hmask", [P, HL], BF16, isOutput=False)
    out_ext = nc.declare_dram_parameter("out", [N // NCORES, D], F32, isOutput=True)

    # A2A bounce buffers (collectives can't touch I/O tensors).
    a2a_in = nc.dram_tensor("a2a_in", [2, NCORES, P, P], BF16)
    a2a_out = nc.dram_tensor("a2a_out", [2, NCORES, P, P], BF16)
    rs_dram = nc.dram_tensor("rs_dram", [3, HL, N], BF16)
    rd_dram = nc.dram_tensor("rd_dram", [4, 1, 1024], F32)

    with tile.TileContext(nc) as tc, \
            tc.tile_pool(name="singles", bufs=1) as singles:

        # ---------------- static SBUF loads ----------------
        def load_w(param):
            t = singles.tile([P, 8, DL], BF16, tag=param.name + "_sb")
            nc.sync.dma_start(out=t, in_=param.rearrange("(f p) c -> p f c", p=P))
            return t

        wq_sb, wk_sb, wv_sb, wck_sb, wcv_sb = (
            load_w(w) for w in (wq, wk, wv, wck, wcv))

        wo_sb = singles.tile([P, 8, D], BF16)
        nc.sync.dma_start(out=wo_sb, in_=wo.rearrange("(f p) c -> p f c", p=P))
        bo_sb = singles.tile([1, D], BF16)
        nc.sync.dma_start(out=bo_sb, in_=bo[0:1, :])

        hmask_sb = singles.tile([P, HL], BF16)
        nc.sync.dma_start(out=hmask_sb, in_=hmask[:, :])

        ones1 = singles.tile([1, P], BF16)
        nc.vector.memset(ones1, 1.0)
        onesb = singles.tile([P, 512], BF16)
        nc.vector.memset(onesb, 1.0)
        eps2 = singles.tile([HL, 1], F32)
        nc.vector.memset(eps2, EPS)

        # Normed/roped activations in transposed layout.
        qTn = singles.tile([P, N], BF16)
        kTn = singles.tile([P, NK], BF16)
        # V in natural layout [keys, dims], 130 = [h0 64 | 1 | h1 64 | 1].
        v_all = singles.tile([P, NK // P, 130], BF16)
        nc.gpsimd.memset(v_all, 1.0)
        # Attention output (normalized), transposed layout.
        oT = singles.tile([P, N], BF16)

        # ---------------- phase 1: projections + RMSNorm + RoPE ----------------
        with tc.tile_pool(name="proj_ps", bufs=4, space="PSUM") as proj_ps, \
                tc.tile_pool(name="ssq_ps", bufs=2, space="PSUM") as ssq_ps, \
                tc.tile_pool(name="vps", bufs=2, space="PSUM") as vps, \
                tc.tile_pool(name="p1big", bufs=1) as p1big, \
                tc.tile_pool(name="rope", bufs=2) as rope, \
                tc.tile_pool(name="p1work", bufs=4) as p1work, \
                tc.tile_pool(name="p1small", bufs=2) as p1small:

            xT_sb = p1big.tile([P, 8, N], BF16)       # 8 f-tiles
            yT_sb = p1big.tile([P, 8, M], BF16)
            nc.sync.dma_start(out=xT_sb, in_=xT.rearrange("(f p) n -> p f n", p=P))
            nc.sync.dma_start(out=yT_sb, in_=yT.rearrange("(f p) n -> p f n", p=P))
            cq_sb = p1big.tile([P, N], BF16)
            sq_sb = p1big.tile([P, N], BF16)
            ckc_sb = p1big.tile([P, NK], BF16)
            cks_sb = p1big.tile([P, NK], BF16)
            nc.sync.dma_start(out=cq_sb, in_=cq[:, :])
            nc.sync.dma_start(out=sq_sb, in_=sq[:, :])
            nc.sync.dma_start(out=ckc_sb, in_=ckc[:, :])
            nc.sync.dma_start(out=cks_sb, in_=cks[:, :])

            rs_slot = [0]

            def qk_proj(w_sb, src_sb, dst, dst_off, npos, c_sb, s_sb, tab_off):
                """Project (transposed), rmsnorm, rope -> dst[:, dst_off:+npos]."""
                nchunk = npos // 512
                t1 = rope.tile([P, npos], BF16, name="t1", tag="t1",
                               padded_shape=[P, NK // 2])
                rs_sb = p1small.tile([HL, npos], BF16, name="rs_sb", tag="rs_sb",
                                     padded_shape=[HL, NK // 2], bufs=2)
                for t in range(nchunk):
                    cs = slice(t * 512, (t + 1) * 512)
                    ps = proj_ps.tile([P, 512], F32)
                    for f in range(8):
                        nc.tensor.matmul(ps, w_sb[:, f, :],
                                         src_sb[:, f, t * 512:(t + 1) * 512],
                                         start=(f == 0), stop=(f == 7))
                    # plain evict (raw, un-normalized)
                    nc.vector.tensor_mul(t1[:, cs], ps, onesb)
                    # squares on DVE from SBUF bf16 (keeps ACT on one table set)
                    qsq = p1work.tile([P, 512], BF16, tag="qsq")
                    nc.vector.tensor_mul(qsq, t1[:, cs], t1[:, cs])
                    # mean-square per head via mask matmul (hmask carries 1/64)
                    ssq = ssq_ps.tile([HL, 512], F32)
                    nc.tensor.matmul(ssq, hmask_sb, qsq, start=True, stop=True)
                    # rs = exp(-0.5 * ln(ms + eps)); single ACT table set
                    lns = p1small.tile([HL, 512], F32, tag="lns")
                    nc.scalar.activation(out=lns, in_=ssq,
                                         func=mybir.ActivationFunctionType.Ln,
                                         bias=eps2)
                    nc.scalar.activation(out=rs_sb[:, cs], in_=lns,
                                         func=mybir.ActivationFunctionType.Exp,
                                         scale=-0.5)
                # one DRAM bounce + per-head partition broadcast for the proj
                slot = rs_slot[0]
                rs_slot[0] += 1
                nc.sync.dma_start(out=rs_dram[slot, :, 0:npos], in_=rs_sb)
                rsb = p1work.tile([P, npos], BF16, name="rsb", tag="rsbw",
                                  padded_shape=[P, NK // 2], bufs=2)
                for h in range(HL):
                    hap = rs_dram[slot, h:h + 1, 0:npos]
                    bsrc = bass.AP(tensor=hap.tensor, offset=hap.offset,
                                   ap=[[0, DH]] + hap.ap[1:])
                    nc.sync.dma_start(out=rsb[h * DH:(h + 1) * DH, :], in_=bsrc)
                # rope over the full row block
                sl = slice(dst_off, dst_off + npos)
                tab = slice(tab_off, tab_off + npos)
                m1 = rope.tile([P, npos], BF16, name="m1", tag="m1",
                               padded_shape=[P, NK // 2])
                nc.vector.tensor_mul(m1, t1, c_sb[:, tab])
                # rotate-half across partitions via SBUF->SBUF DMA (engine-free)
                t1r = rope.tile([P, npos], BF16, name="t1r", tag="t1r",
                                padded_shape=[P, NK // 2])
                for h in range(HL):
                    b = h * DH
                    nc.sync.dma_start(out=t1r[b:b + 32, :], in_=t1[b + 32:b + 64, :])
                    nc.sync.dma_start(out=t1r[b + 32:b + 64, :], in_=t1[b:b + 32, :])
                r1 = rope.tile([P, npos], BF16, name="r1", tag="r1",
                               padded_shape=[P, NK // 2])
                nc.vector.tensor_mul(r1, t1r, s_sb[:, tab])
                s2 = rope.tile([P, npos], BF16, name="s2", tag="t1r",
                               padded_shape=[P, NK // 2])
                nc.vector.tensor_add(s2, m1, r1)
                nc.vector.tensor_mul(dst[:, sl], s2, rsb)

            qk_proj(wq_sb, xT_sb, qTn, 0, N, cq_sb, sq_sb, 0)
            qk_proj(wk_sb, xT_sb, kTn, 0, N, ckc_sb, cks_sb, 0)
            qk_proj(wck_sb, yT_sb, kTn, N, M, ckc_sb, cks_sb, N)

            # V / CV: natural layout, stationary = data chunk.
            for t in range(NK // P):
                src_sb = xT_sb if t < N // P else yT_sb
                w_sb = wv_sb if t < N // P else wcv_sb
                tt = t if t < N // P else t - N // P
                ps = vps.tile([P, DL], F32)
                for f in range(8):
                    nc.tensor.matmul(ps, src_sb[:, f, tt * P:(tt + 1) * P],
                                     w_sb[:, f, :], start=(f == 0), stop=(f == 7))
                # evict into [h0 64 | (1) | h1 64 | (1)] layout, skipping ones cols
                nc.vector.tensor_mul(v_all[:, t, 0:64], ps[:, 0:64], onesb[:, 0:64])
                nc.vector.tensor_mul(v_all[:, t, 65:129], ps[:, 64:128],
                                     onesb[:, 0:64])

        # ---------------- phase 2: attention ----------------
        with tc.tile_pool(name="st_ps", bufs=2, space="PSUM") as st_ps, \
                tc.tile_pool(name="pv_ps", bufs=1, space="PSUM") as pv_ps, \
                tc.tile_pool(name="p2work", bufs=3) as p2work, \
                tc.tile_pool(name="p2small", bufs=2) as p2small:
            for qh in range(2):          # q halves of 1024
                qsl = slice(qh * 1024, (qh + 1) * 1024)
                pv = [pv_ps.tile([65, 1024], F32, name=f"pv{h}", tag=f"pv{h}")
                      for h in range(HL)]
                for kc in range(NK // P):
                    es = []
                    for h in range(HL):
                        hs = slice(h * DH, (h + 1) * DH)
                        st = st_ps.tile([P, 1024], F32, name="st", tag="st")
                        for c in range(2):
                            nc.tensor.matmul(
                                st[:, c * 512:(c + 1) * 512],
                                kTn[hs, kc * P:(kc + 1) * P],
                                qTn[hs, qh * 1024 + c * 512: qh * 1024 + (c + 1) * 512],
                                start=True, stop=True)
                        e = p2work.tile([P, 1024], BF16, name="es", tag="es")
                        nc.scalar.activation(out=e, in_=st,
                                             func=mybir.ActivationFunctionType.Exp,
                                             scale=SCALE)
                        es.append(e)
                    for h in range(HL):
                        for c in range(2):
                            nc.tensor.matmul(
                                pv[h][:, c * 512:(c + 1) * 512],
                                v_all[:, kc, h * 65:(h + 1) * 65],
                                es[h][:, c * 512:(c + 1) * 512],
                                start=(kc == 0), stop=(kc == NK // P - 1))
                for h in range(HL):
                    rd = p2small.tile([1, 1024], F32, tag="rd")
                    nc.vector.reciprocal(rd, pv[h][64:65, :])
                    slot = qh * HL + h
                    nc.sync.dma_start(out=rd_dram[slot, :, :], in_=rd)
                    rdb = p2small.tile([DH, 1024], F32, tag="rdb")
                    hap = rd_dram[slot, 0:1, :]
                    bsrc = bass.AP(tensor=hap.tensor, offset=hap.offset,
                                   ap=[[0, DH]] + hap.ap[1:])
                    nc.sync.dma_start(out=rdb, in_=bsrc)
                    nc.vector.tensor_mul(oT[h * DH:(h + 1) * DH, qsl],
                                         pv[h][0:64, :], rdb)
                # A2A for this q-half: shard j = 128 positions for dest core j.
                # Core j ends up owning rows {j*128..}+{1024+j*128..}.
                for j in range(NCORES):
                    nc.sync.dma_start(
                        out=a2a_in[qh, j, :, :],
                        in_=oT[:, qh * 1024 + j * P: qh * 1024 + (j + 1) * P])
                nc.gpsimd.collective_compute(
                    "AllToAll", mybir.AluOpType.bypass,
                    replica_groups=[list(range(NCORES))],
                    ins=[a2a_in[qh]],
                    outs=[a2a_out[qh]],
                )

        with tc.tile_pool(name="p3", bufs=1) as p3, \
                tc.tile_pool(name="z_ps", bufs=2, space="PSUM") as z_ps, \
                tc.tile_pool(name="zout", bufs=2) as zout:
            for qh in range(2):
                of_sb = p3.tile([P, NCORES, P], BF16, name="of_sb", tag=f"of{qh}")
                for j in range(NCORES):
                    nc.sync.dma_start(out=of_sb[:, j, :], in_=a2a_out[qh, j, :, :])
                for nn in range(2):  # 2 output col chunks of 512
                    zp = z_ps.tile([P, 512], F32)
                    for j in range(NCORES):
                        nc.tensor.matmul(zp, of_sb[:, j, :],
                                         wo_sb[:, j, nn * 512:(nn + 1) * 512],
                                         start=(j == 0), stop=False)
                    nc.tensor.matmul(zp, ones1, bo_sb[:, nn * 512:(nn + 1) * 512],
                                     start=False, stop=True)
                    zs = zout.tile([P, 512], F32)
                    nc.vector.tensor_mul(zs, zp, onesb)
                    nc.sync.dma_start(out=out_ext[qh * P:(qh + 1) * P,
                                                  nn * 512:(nn + 1) * 512],
                                      in_=zs)
    return nc


def _bf16(a):
    return np.ascontiguousarray(a).astype(ml_dtypes.bfloat16)


def _rope_tables(npos, pos0, g_first, g_second, n_first):
    """Tables [128, npos] for transposed-layout rope with g folded in.

    Row j (within a head, duplicated for 2 local heads):
      out[j] = t[j]*C[j] + t[sigma(j)]*S[j]
      j <  32: C[j]=g[j]*cos[n,j],     S[j]=-g[j+32]*sin[n,j]
      j >= 32: C[j]=g[j]*cos[n,j-32],  S[j]=+g[j-32]*sin[n,j-32]
    g switches from g_first to g_second at position n_first.
    """
    inv = 1.0 / (ROPE_BASE ** (np.arange(0, DH, 2, dtype=np.float64) / DH))
    pos = np.arange(pos0, pos0 + npos, dtype=np.float64)
    ang = pos[:, None] * inv[None, :]          # [npos, 32]
    cos = np.cos(ang).T                         # [32, npos]
    sin = np.sin(ang).T
    C = np.zeros((DH, npos), np.float64)
    S = np.zeros((DH, npos), np.float64)
    g = np.zeros((DH, npos), np.float64)
    g[:, :n_first] = np.asarray(g_first, np.float64)[:, None]
    if n_first < npos:
        g[:, n_first:] = np.asarray(g_second, np.float64)[:, None]
    C[:32] = cos
    C[32:] = cos
    C *= g
    S[:32] = -sin
    S[32:] = sin
    Srot = np.concatenate([g[32:], g[:32]], axis=0)  # g[sigma(j)]
    S *= Srot
    C2 = np.concatenate([C, C], axis=0)  # duplicate for 2 local heads
    S2 = np.concatenate([S, S], axis=0)
    return _bf16(C2), _bf16(S2)


_NC_CACHE = None


def kernel(x, y, W_qkv, W_ckv, W_out, b_out, g_q, g_k, g_ck, n_heads):
    global LAST_RESULT, _NC_CACHE
    x = np.asarray(x, np.float32)
    y = np.asarray(y, np.float32)
    W_qkv = np.asarray(W_qkv, np.float32)
    W_ckv = np.asarray(W_ckv, np.float32)
    W_out = np.asarray(W_out, np.float32)
    b_out = np.asarray(b_out, np.float32)

    xT = _bf16(x[0].T)                       # [1024, 2048]
    yT = _bf16(y[0].T)
    Wq, Wk, Wv = (W_qkv[:, i * D:(i + 1) * D] for i in range(3))
    Wck, Wcv = (W_ckv[:, i * D:(i + 1) * D] for i in range(2))
    woh = _bf16(W_out)
    boh = _bf16(b_out[None, :])

    cqh, sqh = _rope_tables(N, 0, g_q, g_q, N)
    ckch, cksh = _rope_tables(NK, 0, g_k, g_ck, N)
    hm = np.zeros((P, HL), np.float32)
    for h in range(HL):
        hm[h * DH:(h + 1) * DH, h] = 1.0 / DH
    hmh = _bf16(hm)

    in_maps = []
    for c in range(NCORES):
        sl = slice(c * DL, (c + 1) * DL)
        in_maps.append({
            "xT": xT, "yT": yT,
            "wq": _bf16(Wq[:, sl]), "wk": _bf16(Wk[:, sl]),
            "wv": _bf16(Wv[:, sl]), "wck": _bf16(Wck[:, sl]),
            "wcv": _bf16(Wcv[:, sl]),
            "wo": woh, "bo": boh,
            "cq": cqh, "sq": sqh, "ckc": ckch, "cks": cksh,
            "hmask": hmh,
        })

    if _NC_CACHE is None:
        _NC_CACHE = build_nc()
        if not _NC_CACHE.is_finalized():
            _NC_CACHE.finalize()
    nc = _NC_CACHE

    res = run_bass_kernel_spmd(
        nc, in_maps, core_ids=list(range(NCORES)),
        trace=bool(os.environ.get("BASS_TRACE")),
    )
    LAST_RESULT = res
    out = np.empty((N, D), np.float32)
    for c in range(NCORES):
        o = np.asarray(res.results[c]["out"], np.float32)
        out[c * P:(c + 1) * P] = o[0:P]
        out[N // 2 + c * P:N // 2 + (c + 1) * P] = o[P:2 * P]
    return out[None, :, :]

